# revision 1
# baseline (speedup 1.0000x reference)
"""GAT-style graph-attention kernel for Trainium2, sharded over 8 NeuronCores.

Math (reference):
  h = x*conv_w + conv_b                       [N, D]
  Wh1 = h @ a1.T ; Wh2 = h @ a2.T             [N, H]
  e[k,i,j] = elu(Wh1[i,k] + Wh2[j,k])
  att = softmax_j(where(adj>0, e, -9e15))
  out = elu(0.5*mean_k(att@h) + 0.5*h); out /= max(||out||_2, 1e-12); out += bias

Key identities used on device:
  q := exp(z) = exp(w1_i)*exp(w2_j)  (outer product of tiny exp vectors)
  exp(elu(z)) = min(exp(q-1), max(q, 1))   (exact; one ACT pass per element)
  masked scores: p = exp(elu(z))*mask; softmax denominator via a ones-column
  appended to h in the att@h matmul (rowsum rides the matmul for free).

Sharding: each core owns a 512-row block of the output for ALL 4 heads
(row-parallel; no collectives). Scores are built transposed (j on
partitions) so the att@h matmul needs no transposes; the host passes a
transposed adjacency slice per core.
"""
import sys

if "/opt/trn_rl_repo" not in sys.path:
    sys.path.insert(0, "/opt/trn_rl_repo")

import numpy as np
from contextlib import ExitStack

import concourse.bass as bass
import concourse.tile as tile
from concourse import bacc, mybir

N, D, H = 4096, 256, 4
NCORES = 8
R = N // NCORES          # 512 rows per core
JT = N // 128            # 32 j-tiles
IC = R // 128            # 4 i-chunks per core
SB = 2                   # j-tiles per superblock
NSB = JT // SB           # 16 superblocks
HP = 2                   # heads per head-pair sweep
WID = HP * SB * R        # free width of a score tile (2*4*512 = 4096)

_QMOD = 4                         # 1-in-_QMOD of q ops run on DVE
_BUFS = [3, 3, 2, 2, 3]           # bufs for q/u/g/p0/p score pools

FP32 = mybir.dt.float32
BF16 = mybir.dt.bfloat16
I32 = mybir.dt.int32
AF = mybir.ActivationFunctionType
ALU = mybir.AluOpType


def _build_program(w_conv: float, b_conv: float):
    nc = bacc.Bacc("TRN2", target_bir_lowering=False, debug=False,
                   num_devices=NCORES)

    x_d = nc.dram_tensor("x", [N, D], FP32, kind="ExternalInput")
    xI_d = nc.dram_tensor("xI", [R, D], FP32, kind="ExternalInput")
    xT_d = nc.dram_tensor("xT", [D, N], FP32, kind="ExternalInput")
    xTI_d = nc.dram_tensor("xTI", [D, R], FP32, kind="ExternalInput")
    a8_d = nc.dram_tensor("a8", [D, 2 * H], FP32, kind="ExternalInput")
    adjT_d = nc.dram_tensor("adjT", [N, R], mybir.dt.uint8,
                        kind="ExternalInput")
    bias_d = nc.dram_tensor("bias", [1, D], FP32, kind="ExternalInput")
    out_d = nc.dram_tensor("out", [R, D], FP32, kind="ExternalOutput")

    with tile.TileContext(nc) as tc, ExitStack() as ctx:
        per = ctx.enter_context(tc.tile_pool(name="per", bufs=1))
        # mask quads: [128, SB*R] bf16, one per superblock, layout (jl, i)
        maskQ = [per.tile([128, SB * R], BF16, tag=f"maskQ{qb}",
                          name=f"maskQ{qb}") for qb in range(NSB)]
        h_aug = [per.tile([128, D + 1], BF16, tag=f"h_aug{jb}",
                          name=f"h_aug{jb}") for jb in range(JT)]
        qwh = [per.tile([128, 4 * 2 * H], FP32, tag=f"qwh{g}", name=f"qwh{g}")
               for g in range(JT // 4)]
        qv1bc = per.tile([128, H * R], BF16, tag="qv1bc")
        h_I = per.tile([128, IC * D], FP32, tag="h_I")           # 0.5*h own rows
        accp = [per.tile([128, D], FP32, tag=f"accp{icc}", name=f"accp{icc}")
                for icc in range(IC)]
        bias_bc = per.tile([128, D], FP32, tag="bias_bc")
        neg1 = per.tile([128, 1], FP32, tag="neg1")
        nc.gpsimd.memset(neg1[:], -1.0)

        # ---------------- main-loop pools (allocated first so their SBUF
        # does not overlap freed setup pools, which would serialize) -------
        qp = ctx.enter_context(tc.tile_pool(name="q", bufs=_BUFS[0]))
        up = ctx.enter_context(tc.tile_pool(name="u", bufs=_BUFS[1]))
        gp = ctx.enter_context(tc.tile_pool(name="g", bufs=_BUFS[2]))
        p0p = ctx.enter_context(tc.tile_pool(name="p0", bufs=_BUFS[3]))
        pp = ctx.enter_context(tc.tile_pool(name="p", bufs=_BUFS[4]))
        ep = ctx.enter_context(tc.tile_pool(name="ep", bufs=4))

        # ---------------- setup ----------------
        stg = ctx.enter_context(tc.tile_pool(name="stage", bufs=4))
        with tc.tile_pool(name="setup", bufs=1) as sp, \
             tc.tile_pool(name="pwI", bufs=1, space="PSUM") as pwI_pool, \
             tc.tile_pool(name="pw", bufs=4, space="PSUM") as pw_pool:
            # qv1 path first (small, unblocks the main loop quickly).
            # h = x*w+b is folded into the Wh matmuls: Wh = w*(X@a8) + b*S,
            # S = colsum(a8); the b*S terms for BOTH halves ride qv1's exp
            # bias (q = e^{w*PW1} * e^{w*PW2} * e^{b*(S_k+S_{H+k})}).
            a8t = []
            for dc in range(2):
                a8c = sp.tile([128, 2 * H], FP32, tag=f"a8{dc}",
                              name=f"a8{dc}")
                nc.sync.dma_start(a8c[:], a8_d[dc * 128:(dc + 1) * 128, :])
                a8t.append(a8c)
            ones_col = sp.tile([128, 1], FP32, tag="ones_col")
            nc.gpsimd.memset(ones_col[:], 1.0)
            pS = pwI_pool.tile([2 * H, 1], FP32, tag="pS")
            for dc in range(2):
                nc.tensor.matmul(pS[:], a8t[dc][:], ones_col[:],
                                 start=(dc == 0), stop=(dc == 1))
            S12 = sp.tile([2 * H, 1], FP32, tag="S12")
            nc.vector.tensor_copy(S12[:], pS[:])
            Shi = sp.tile([H, 1], FP32, tag="Shi")
            nc.sync.dma_start(Shi[:], S12[H:2 * H, :])
            qbias = sp.tile([H, 1], FP32, tag="qbias")
            nc.vector.tensor_add(qbias[:], S12[0:H, :], Shi[:])
            nc.vector.tensor_scalar(qbias[:], qbias[:], b_conv, None,
                                    op0=ALU.mult)
            xtiI = []
            for dc in range(2):
                xti = sp.tile([128, R], FP32, tag=f"xTI{dc}", name=f"xTI{dc}")
                nc.sync.dma_start(xti[:], xTI_d[dc * 128:(dc + 1) * 128, :])
                xtiI.append(xti)
            pwI = pwI_pool.tile([2 * H, R], FP32, tag="pwI")
            for dc in range(2):
                nc.tensor.matmul(pwI[:], a8t[dc][:], xtiI[dc][:],
                                 start=(dc == 0), stop=(dc == 1))
            qwhTI = sp.tile([2 * H, R], BF16, tag="qwhTI")
            nc.scalar.activation(qwhTI[0:H, :], pwI[0:H, :], AF.Exp,
                                 bias=qbias[:], scale=w_conv)
            qv1row = sp.tile([1, H * R], BF16, tag="qv1row")
            nc.sync.dma_start(
                qv1row[:].rearrange("o (k i) -> o k i", k=H), qwhTI[0:H, :])
            nc.gpsimd.partition_broadcast(qv1bc[:], qv1row[:])

            # Wh2[j,:] for all j from raw x^T chunks (exp applies w_conv)
            for grp in range(JT // 4):
                hTg = []
                for dc in range(2):
                    xtc = stg.tile([128, 512], FP32, tag="xtc",
                                   name=f"xtc{grp}_{dc}")
                    nc.sync.dma_start(
                        xtc[:], xT_d[dc * 128:(dc + 1) * 128,
                                     grp * 512:(grp + 1) * 512])
                    hTg.append(xtc)
                pw = pw_pool.tile([128, 4 * 2 * H], FP32, tag="pw",
                                  name=f"pw{grp}")
                for jb in range(grp * 4, grp * 4 + 4):
                    off = (jb % 4) * 128
                    col = (jb % 4) * 2 * H
                    for dc in range(2):
                        nc.tensor.matmul(
                            pw[:, col:col + 2 * H],
                            hTg[dc][:, off:off + 128], a8t[dc][:],
                            start=(dc == 0), stop=(dc == 1))
                nc.scalar.activation(qwh[grp][:], pw[:], AF.Exp, scale=w_conv)

            bias_row = sp.tile([1, D], FP32, tag="bias_row")
            nc.sync.dma_start(bias_row[:], bias_d[:, :])
            nc.gpsimd.partition_broadcast(bias_bc[:], bias_row[:])

            # h_I = 0.5*h for own rows
            for icc in range(IC):
                xi = stg.tile([128, D], FP32, tag="xistg", name=f"xi{icc}")
                nc.sync.dma_start(xi[:], xI_d[icc * 128:(icc + 1) * 128, :])
                nc.scalar.activation(
                    h_I[:, icc * D:(icc + 1) * D], xi[:], AF.Copy,
                    bias=0.5 * b_conv, scale=0.5 * w_conv)


        # ---------------- main: score tiles + matmul ----------------
        # score tile free layout: (head-local, j-tile, i) = [HP, SB, R]
        with tc.tile_pool(name="pm", bufs=1, space="PSUM") as pmp:
            for hp in range(2):
                heads = (2 * hp, 2 * hp + 1)
                pm = [[pmp.tile([128, D + 1], FP32, tag=f"pm{hl}{icc}",
                                name=f"pm{hl}{icc}_{hp}")
                       for icc in range(IC)] for hl in range(HP)]
                for sb in range(NSB):
                    jb0 = SB * sb
                    if hp == 0:
                        # stream in this superblock's adjacency
                        for jb in range(jb0, jb0 + SB):
                            at = stg.tile([128, R], mybir.dt.uint8,
                                          tag="astg", name=f"at{jb}")
                            nc.scalar.dma_start(
                                at[:], adjT_d[jb * 128:(jb + 1) * 128, :])
                            nc.gpsimd.tensor_copy(
                                maskQ[jb // SB][:, (jb % SB) * R:
                                                (jb % SB + 1) * R], at[:])
                    q = qp.tile([128, WID], BF16, tag="q")
                    u = up.tile([128, WID], BF16, tag="u")
                    g = gp.tile([128, WID], BF16, tag="g")
                    p0 = p0p.tile([128, WID], BF16, tag="p0")
                    p = pp.tile([128, WID], BF16, tag="p")
                    # q = exp(w1) (x) exp(w2); TS is 4x on DVE -- put 1/4
                    # of the ops on DVE, the rest on GPSIMD for balance
                    for hl in range(HP):
                        for jl in range(SB):
                            sec = (hl * SB + jl) * R
                            qeng = (nc.vector
                                    if (hl * SB + jl + sb) % _QMOD == 0
                                    else nc.gpsimd)
                            jb = jb0 + jl
                            qc = (jb % 4) * 2 * H + H + heads[hl]
                            qeng.tensor_scalar(
                                q[:, sec:sec + R],
                                qv1bc[:, heads[hl] * R:(heads[hl] + 1) * R],
                                qwh[jb // 4][:, qc:qc + 1],
                                None, op0=ALU.mult)
                    # u = exp(q - 1); p0 = min(u, max(q, 1))  [exact]
                    nc.scalar.activation(u[:], q[:], AF.Exp, bias=neg1[:])
                    nc.vector.tensor_scalar(g[:], q[:], 1.0, None, op0=ALU.max)
                    nc.vector.tensor_tensor(p0[:], u[:], g[:], op=ALU.min)
                    # p = p0 * mask (flat; mask broadcast across the 2 heads)
                    p03 = p0[:].rearrange("p (h ji) -> p h ji", h=HP)
                    p3 = p[:].rearrange("p (h ji) -> p h ji", h=HP)
                    mrep = (maskQ[sb][:].unsqueeze(1)
                            .to_broadcast([128, HP, SB * R]))
                    nc.vector.tensor_tensor(p3, p03, mrep, op=ALU.mult)
                    if hp == 0:
                        # h rows for this superblock (needed by the matmuls)
                        for jb in range(jb0, jb0 + SB):
                            xt = stg.tile([128, D], FP32, tag="xstg",
                                          name=f"xs{jb}")
                            nc.sync.dma_start(
                                xt[:], x_d[jb * 128:(jb + 1) * 128, :])
                            nc.gpsimd.memset(h_aug[jb][:, D:D + 1], 1.0)
                            nc.scalar.activation(
                                h_aug[jb][:, 0:D], xt[:], AF.Copy,
                                bias=b_conv, scale=w_conv)
                    # accumulate p^T @ [h|1] over j into PSUM
                    for jl in range(SB):
                        rhs = h_aug[jb0 + jl][:]
                        for hl in range(HP):
                            for icc in range(IC):
                                sec = (hl * SB + jl) * R + icc * 128
                                nc.tensor.matmul(
                                    pm[hl][icc][:], p[:, sec:sec + 128], rhs,
                                    start=(sb == 0 and jl == 0),
                                    stop=(sb == NSB - 1 and jl == SB - 1))
                # fold this head-pair into accp: accp += pm[:, :D] / s
                for hl in range(HP):
                    for icc in range(IC):
                        rcp = ep.tile([128, 1], FP32, tag="rcp")
                        nc.vector.reciprocal(rcp[:], pm[hl][icc][:, D:D + 1])
                        acs = accp[icc][:]
                        if hp == 0 and hl == 0:
                            nc.vector.tensor_scalar(
                                acs, pm[hl][icc][:, :D], rcp[:], None,
                                op0=ALU.mult)
                        else:
                            nc.vector.scalar_tensor_tensor(
                                acs, pm[hl][icc][:, :D], rcp[:], acs,
                                op0=ALU.mult, op1=ALU.add)

            # ---------------- epilogue ----------------
            for icc in range(IC):
                acs = accp[icc][:]
                t = ep.tile([128, D], FP32, tag="t")
                # t = 0.125*acc + 0.5*h   (h_I already holds 0.5*h)
                nc.vector.scalar_tensor_tensor(
                    t[:], acs, 0.125, h_I[:, icc * D:(icc + 1) * D],
                    op0=ALU.mult, op1=ALU.add)
                # elu(t) = relu(t) + min(exp(t), 1) - 1
                eq = ep.tile([128, D], FP32, tag="eq")
                nc.scalar.activation(eq[:], t[:], AF.Exp)
                o1 = ep.tile([128, D], FP32, tag="o1")
                nc.vector.tensor_scalar(o1[:], eq[:], 1.0, -1.0,
                                        op0=ALU.min, op1=ALU.add)
                o = ep.tile([128, D], FP32, tag="o")
                nc.vector.scalar_tensor_tensor(o[:], t[:], 0.0, o1[:],
                                               op0=ALU.max, op1=ALU.add)
                # row L2 norm
                sq = ep.tile([128, D], FP32, tag="sq")
                ss = ep.tile([128, 1], FP32, tag="ss")
                nc.vector.tensor_mul(sq[:], o[:], o[:])
                nc.vector.tensor_reduce(ss[:], sq[:],
                                        axis=mybir.AxisListType.X, op=ALU.add)
                nrm = ep.tile([128, 1], FP32, tag="nrm")
                nc.scalar.activation(nrm[:], ss[:], AF.Sqrt)
                nrm2 = ep.tile([128, 1], FP32, tag="nrm2")
                nc.vector.tensor_scalar(nrm2[:], nrm[:], 1e-12, None,
                                        op0=ALU.max)
                rcpn = ep.tile([128, 1], FP32, tag="rcpn")
                nc.vector.reciprocal(rcpn[:], nrm2[:])
                outv = ep.tile([128, D], FP32, tag="outv")
                nc.vector.scalar_tensor_tensor(
                    outv[:], o[:], rcpn[:], bias_bc[:],
                    op0=ALU.mult, op1=ALU.add)
                nc.sync.dma_start(out_d[icc * 128:(icc + 1) * 128, :], outv[:])

    nc.finalize()
    return nc


_PROGRAM_CACHE = {}


def _get_program(w_conv: float, b_conv: float):
    key = (w_conv, b_conv)
    if key not in _PROGRAM_CACHE:
        _PROGRAM_CACHE[key] = _build_program(w_conv, b_conv)
    return _PROGRAM_CACHE[key]


def kernel(x, adj, conv_w, conv_b, a, bias, _want_results=False, _trace=False,
           **_ignored):
    from concourse.bass_utils import run_bass_kernel_spmd

    x = np.asarray(x, dtype=np.float32)
    adj = np.ascontiguousarray(np.asarray(adj, dtype=np.int32))
    a = np.asarray(a, dtype=np.float32)
    bias = np.asarray(bias, dtype=np.float32)
    w_conv = float(np.asarray(conv_w).reshape(-1)[0])
    b_conv = float(np.asarray(conv_b).reshape(-1)[0])

    xn = np.ascontiguousarray(x.reshape(N, D))
    xT = np.ascontiguousarray(xn.T)
    a1 = a[:, :D, 0]
    a2 = a[:, D:, 0]
    a8 = np.ascontiguousarray(np.concatenate([a1, a2], axis=0).T)  # [D, 2H]
    bias_row = np.ascontiguousarray(bias.reshape(1, D))

    nc = _get_program(w_conv, b_conv)

    in_maps = []
    for c in range(NCORES):
        rows = slice(c * R, (c + 1) * R)
        in_maps.append({
            "x": xn,
            "xI": np.ascontiguousarray(xn[rows]),
            "xT": xT,
            "xTI": np.ascontiguousarray(xT[:, rows]),
            "a8": a8,
            "adjT": np.ascontiguousarray(adj[rows].T).astype(np.uint8),
            "bias": bias_row,
        })

    res = run_bass_kernel_spmd(nc, in_maps, core_ids=list(range(NCORES)),
                               trace=_trace)
    out = np.concatenate([res.results[c]["out"] for c in range(NCORES)], axis=0)
    if _want_results:
        return out, res
    return out

